# revision 6
# baseline (speedup 1.0000x reference)
"""Trainium2 Bass kernel for nn_FNO_RC_1D (1D FNO + Chebyshev-Fourier residual
correction). Data-parallel over batch: 32 samples -> 8 cores x 4 samples.

Factorization (per sample, h is [128 ch, 8192 s] bf16 in SBUF):
  - fc0 as K=2 matmul vs [x; grid]
  - spectral conv: rfft truncated to 32 modes == h @ F (F: [S, 64] cos/-sin),
    done as 64 chunk-matmuls with lhsT = hT chunks (DMA xbar transpose of h),
    per-mode complex mixing as 64 small matmuls (sw weights stationary,
    staged xf moving, N=8), irfft of 32 modes == ofT.T @ Cinv ([64, S]);
    inverse-DFT and 1x1-conv matmuls accumulate into the same PSUM so
    bias + GELU is a single ScalarE pass PSUM->SBUF.
  - CFT/latent path computed exactly: 72 extra basis columns ride along the
    layer-3 forward DFT; cg1_w is folded over the L-broadcast on host.
  - fc1 (+GELU) chunk-wise; fc2 transposed (out [s-chunk, 1] per chunk) so the
    final [S] vector lands across partitions.
All matmul operands bf16, accumulation fp32 in PSUM.
"""

from contextlib import ExitStack

import numpy as np
import ml_dtypes

B, S, WIDTH, MODES = 32, 8192, 128, 32
CFT_MODES, L_SEG, M_CHEB = 4, 2, 4
NCORES = 8
BPC = B // NCORES  # samples per core
NCH = S // 128     # 64 chunks
BF = ml_dtypes.bfloat16

_CACHE = {}


def _cheb_basis(n, m):
    t = np.linspace(-1.0, 1.0, n)
    Ts = [np.ones(n), t]
    for _ in range(2, m):
        Ts.append(2.0 * t * Ts[-1] - Ts[-2])
    return np.stack(Ts[:m], 0).astype(np.float32)


def _host_consts():
    s = np.arange(S, dtype=np.float64)
    k = np.arange(MODES, dtype=np.float64)
    ang = 2.0 * np.pi * np.outer(s, k) / S
    F = np.concatenate([np.cos(ang), -np.sin(ang)], axis=1)          # [S, 64]
    ck = np.full(MODES, 2.0 / S); ck[0] = 1.0 / S
    Cinv = np.empty((2 * MODES, S), np.float64)                       # interleaved
    Cinv[0::2] = ck[:, None] * np.cos(ang.T)
    Cinv[1::2] = -ck[:, None] * np.sin(ang.T)
    T = _cheb_basis(S, M_CHEB).astype(np.float64)                     # [4, S]
    kk = np.arange(-CFT_MODES, CFT_MODES + 1, dtype=np.float64)
    ph = np.pi * np.outer(s, kk) / S
    CH = np.empty((S, M_CHEB, 2 * CFT_MODES + 1, 2), np.float64)
    CH[..., 0] = T.T[:, :, None] * np.cos(ph)[:, None, :]
    CH[..., 1] = T.T[:, :, None] * (-np.sin(ph))[:, None, :]
    CH = (CH / S).reshape(S, 72)
    # chunk-major layouts [128 p, 64 t, cols]: row s = t*128 + p
    F_sb = F.reshape(NCH, 128, 64).transpose(1, 0, 2).astype(BF)
    CH_sb = CH.reshape(NCH, 128, 72).transpose(1, 0, 2).astype(BF)
    grid = np.linspace(0.0, 1.0, S, dtype=np.float32)
    return F_sb, CH_sb, Cinv.astype(BF), grid


def _build():
    import concourse.bacc as bacc
    import concourse.tile as tile
    import concourse.mybir as mybir
    from concourse.masks import make_identity

    f32 = mybir.dt.float32
    bf16 = mybir.dt.bfloat16
    GELU = mybir.ActivationFunctionType.Gelu
    IDENT = mybir.ActivationFunctionType.Identity

    nc = bacc.Bacc("TRN2", target_bir_lowering=False)

    # ---- DRAM tensors ----
    d_xg = nc.dram_tensor("xg", [2 * BPC, S], bf16, kind="ExternalInput")
    d_fc0w = nc.dram_tensor("fc0w", [8, 4, 128], bf16, kind="ExternalInput")
    d_F = nc.dram_tensor("Fb", [128, NCH, 64], bf16, kind="ExternalInput")
    d_CH = nc.dram_tensor("CHb", [128, NCH, 72], bf16, kind="ExternalInput")
    d_Ci = nc.dram_tensor("Cinv", [64, S], bf16, kind="ExternalInput")
    d_WT = nc.dram_tensor("WT", [128, 4, 128], bf16, kind="ExternalInput")
    d_SW = nc.dram_tensor("SW", [4, 128, MODES, 2, 128], bf16, kind="ExternalInput")
    d_G = nc.dram_tensor("G2", [128, 72, 256], bf16, kind="ExternalInput")
    d_fc1w = nc.dram_tensor("fc1w", [128, 128], bf16, kind="ExternalInput")
    d_fc2w = nc.dram_tensor("fc2w", [128, 1], bf16, kind="ExternalInput")
    d_cg2h = nc.dram_tensor("cg2h", [128, 2, 128], bf16, kind="ExternalInput")
    d_fc0b = nc.dram_tensor("fc0b", [128, 1], f32, kind="ExternalInput")
    d_lb = nc.dram_tensor("lb", [128, 3], f32, kind="ExternalInput")     # w0..w2 bias
    d_w3b = nc.dram_tensor("w3b", [128, 1], f32, kind="ExternalInput")   # w3_b+cg2_b
    d_fc1b = nc.dram_tensor("fc1b", [128, 1], f32, kind="ExternalInput")
    d_cg1b = nc.dram_tensor("cg1b", [4, 256], f32, kind="ExternalInput")
    d_out = nc.dram_tensor("out", [BPC, S], f32, kind="ExternalOutput")

    with ExitStack() as ctx:
        tc = ctx.enter_context(tile.TileContext(nc))
        consts = ctx.enter_context(tc.tile_pool(name="consts", bufs=1))
        hpool = ctx.enter_context(tc.tile_pool(name="h", bufs=1))
        htp = ctx.enter_context(tc.tile_pool(name="ht", bufs=2))
        swp = ctx.enter_context(tc.tile_pool(name="sw", bufs=2))
        gp = ctx.enter_context(tc.tile_pool(name="g", bufs=2))
        outp = ctx.enter_context(tc.tile_pool(name="outc", bufs=3))
        stg = ctx.enter_context(tc.tile_pool(name="stg", bufs=1))
        pz = ctx.enter_context(tc.tile_pool(name="pz", bufs=2, space="PSUM"))
        pxf = ctx.enter_context(tc.tile_pool(name="pxf", bufs=1, space="PSUM"))
        pof = ctx.enter_context(tc.tile_pool(name="pof", bufs=1, space="PSUM"))
        psm = ctx.enter_context(tc.tile_pool(name="psm", bufs=1, space="PSUM"))
        pf2 = ctx.enter_context(tc.tile_pool(name="pf2", bufs=1, space="PSUM"))

        sy, gs = nc.sync, nc.gpsimd

        # ---- constants into SBUF ----
        xg = consts.tile([2 * BPC, S], bf16); sy.dma_start(xg, d_xg[:, :])
        fc0w = consts.tile([8, 4, 128], bf16); sy.dma_start(fc0w, d_fc0w[:, :, :])
        Fb = consts.tile([128, NCH, 64], bf16); sy.dma_start(Fb, d_F[:, :, :])
        CHb = consts.tile([128, NCH, 72], bf16); sy.dma_start(CHb, d_CH[:, :, :])
        Ci = consts.tile([64, S], bf16); sy.dma_start(Ci, d_Ci[:, :])
        WT = consts.tile([128, 4, 128], bf16); sy.dma_start(WT, d_WT[:, :, :])
        fc1w = consts.tile([128, 128], bf16); sy.dma_start(fc1w, d_fc1w[:, :])
        fc2w = consts.tile([128, 1], bf16); sy.dma_start(fc2w, d_fc2w[:, :])
        cg2h = consts.tile([128, 2, 128], bf16); sy.dma_start(cg2h, d_cg2h[:, :, :])
        fc0b = consts.tile([128, 1], f32); sy.dma_start(fc0b, d_fc0b[:, :])
        lb = consts.tile([128, 3], f32); sy.dma_start(lb, d_lb[:, :])
        w3b = consts.tile([128, 1], f32); sy.dma_start(w3b, d_w3b[:, :])
        fc1b = consts.tile([128, 1], f32); sy.dma_start(fc1b, d_fc1b[:, :])
        cg1b = consts.tile([4, 256], f32); sy.dma_start(cg1b, d_cg1b[:, :])
        ident = consts.tile([128, 128], bf16); make_identity(nc, ident)

        hs = [hpool.tile([128, S], bf16, tag=f"h{b}", name=f"h{b}")
              for b in range(BPC)]
        A = consts.tile([128, 256], bf16)      # staged (xr, xi) per (k, b)
        Bs = consts.tile([128, 256], bf16)     # staged (-xi, xr)
        feats = consts.tile([128, 288], bf16)  # cft feats [c, (q, b)]
        ofn = consts.tile([128, 256], bf16)    # of natural copy
        ofTs = [consts.tile([64, 128], bf16, tag=f"ofT{b}", name=f"ofT{b}")
                for b in range(BPC)]
        latb = consts.tile([128, BPC], f32)

        # ---- fc0: h0 = fc0_w.T @ [x; grid] + fc0_b ----
        for b in range(BPC):
            for w in range(8):  # windows of 1024
                zt = pz.tile([128, 1024], f32, tag="z")
                for q in range(2):
                    nc.tensor.matmul(
                        zt[:, q * 512:(q + 1) * 512], fc0w[:, b, :],
                        xg[:, w * 1024 + q * 512:w * 1024 + (q + 1) * 512],
                        start=True, stop=True)
                nc.scalar.activation(hs[b][:, w * 1024:(w + 1) * 1024], zt, IDENT,
                                     bias=fc0b[:, 0:1])

        # ---- layers ----
        for l in range(4):
            sw = swp.tile([128, MODES, 2, 128], bf16, tag="sw")
            gs.dma_start(sw, d_SW[l, :, :, :, :])
            # phase 1: transpose + forward DFT (+ CFT at l==3)
            for b in range(BPC):
                xfp = pxf.tile([128, 136], f32, tag="xf")
                if l == 3:
                    cftp = psm.tile([128, 72], f32, tag="sm")
                for hh in range(2):
                    ht = htp.tile([128, 32, 128], bf16, tag="ht")
                    sy.dma_start(ht, hs[b][:, hh * 4096:(hh + 1) * 4096],
                                 transpose=True)
                    for t in range(32):
                        tg = hh * 32 + t
                        nc.tensor.matmul(xfp[:, 0:64], ht[:, t, :], Fb[:, tg, :],
                                         start=(tg == 0), stop=(tg == 63))
                        if l == 3:
                            nc.tensor.matmul(cftp, ht[:, t, :],
                                             CHb[:, tg, :],
                                             start=(tg == 0), stop=(tg == 63))
                # stage xf -> A/B (bf16, strided col writes), negate xi for B
                nc.vector.tensor_copy(A[:, 2 * b:256:8], xfp[:, 0:32])
                nc.vector.tensor_copy(A[:, 2 * b + 1:256:8], xfp[:, 32:64])
                nc.vector.tensor_copy(Bs[:, 2 * b + 1:256:8], xfp[:, 0:32])
                nc.vector.tensor_scalar_mul(Bs[:, 2 * b:256:8], xfp[:, 32:64], -1.0)
                if l == 3:
                    nc.vector.tensor_copy(feats[:, b:288:4], cftp)

            # phase 2: mode mixing -> of_nat [o, (k, b, re/im)]
            ofp = pof.tile([128, 256], f32, tag="of")
            for k in range(MODES):
                nc.tensor.matmul(ofp[:, 8 * k:8 * k + 8], sw[:, k, 0, :],
                                 A[:, 8 * k:8 * k + 8], start=True, stop=False)
                nc.tensor.matmul(ofp[:, 8 * k:8 * k + 8], sw[:, k, 1, :],
                                 Bs[:, 8 * k:8 * k + 8], start=False, stop=True)
            ofp3 = ofp.rearrange("p (k g) -> p k g", g=8)
            for b in range(BPC):
                # contiguous [128, 64] staging of sample b's (k, re/im) cols
                nc.vector.tensor_copy(ofn[:, 64 * b:64 * (b + 1)],
                                      ofp3[:, :, 2 * b:2 * b + 2])
                otp = psm.tile([64, 128], bf16, tag="sm")
                nc.tensor.transpose(otp, ofn[:, 64 * b:64 * (b + 1)], ident)
                nc.vector.tensor_copy(ofTs[b], otp)

            # latent path (l == 3): needs feats, runs before fno drains
            if l == 3:
                tps = pxf.tile([4, 256], f32, tag="xf")
                for qc in range(9):
                    gt = gp.tile([128, 8, 256], bf16, tag="G")
                    gs.dma_start(gt, d_G[:, qc * 8:(qc + 1) * 8, :])
                    for qq in range(8):
                        q = qc * 8 + qq
                        nc.tensor.matmul(tps, feats[:, 4 * q:4 * q + 4],
                                         gt[:, qq, :],
                                         start=(q == 0), stop=(q == 71))
                tsb = stg.tile([4, 256], f32)
                nc.vector.tensor_add(tsb, tps, cg1b)
                tgb = stg.tile([4, 256], bf16)
                nc.scalar.activation(tgb, tsb, GELU)
                lps = pof.tile([128, BPC], f32, tag="of")
                for hh in range(2):
                    ttp = psm.tile([128, 4], bf16, tag="sm")
                    nc.tensor.transpose(ttp, tgb[:, hh * 128:(hh + 1) * 128],
                                        ident[0:4, 0:4])
                    tgT = stg.tile([128, 4], bf16, tag=f"tgT{hh}")
                    nc.vector.tensor_copy(tgT, ttp)
                    nc.tensor.matmul(lps, cg2h[:, hh, :], tgT,
                                     start=(hh == 0), stop=(hh == 1))
                nc.vector.tensor_scalar_add(latb, lps, w3b[:, 0:1])

            # phase 3: z = invDFT + pointwise; drain (gelu / fno+fc1+fc2)
            for b in range(BPC):
                if l == 3:
                    f2ps = pf2.tile([128, 64], f32, tag="f2")
                for w in range(8):  # windows of 1024
                    zt = pz.tile([128, 1024], f32, tag="z")
                    for q in range(2):
                        sl = slice(w * 1024 + q * 512, w * 1024 + (q + 1) * 512)
                        nc.tensor.matmul(zt[:, q * 512:(q + 1) * 512],
                                         ofTs[b], Ci[:, sl], start=True, stop=False)
                        nc.tensor.matmul(zt[:, q * 512:(q + 1) * 512],
                                         WT[:, l, :], hs[b][:, sl],
                                         start=False, stop=True)
                    if l < 3:
                        nc.scalar.activation(hs[b][:, w * 1024:(w + 1) * 1024], zt,
                                             GELU, bias=lb[:, l:l + 1])
                    else:
                        oc = outp.tile([128, 1024], bf16, tag="oc")
                        nc.vector.tensor_scalar_add(oc, zt, latb[:, b:b + 1])
                        # fc1 + gelu + fc2 chunk-wise
                        fps = pz.tile([128, 1024], f32, tag="z")
                        for q in range(2):
                            nc.tensor.matmul(fps[:, q * 512:(q + 1) * 512], fc1w,
                                             oc[:, q * 512:(q + 1) * 512],
                                             start=True, stop=True)
                        g1 = outp.tile([128, 1024], bf16, tag="g1")
                        nc.scalar.activation(g1, fps, GELU, bias=fc1b[:, 0:1])
                        for q in range(8):
                            tg = w * 8 + q
                            nc.tensor.matmul(f2ps[:, tg:tg + 1],
                                             g1[:, q * 128:(q + 1) * 128], fc2w,
                                             start=True, stop=True)
                if l == 3:
                    f2sb = outp.tile([128, 64], f32, tag="f2sb")
                    nc.vector.tensor_copy(f2sb, f2ps)
                    sy.dma_start(d_out[b, :].rearrange("(t p) -> p t", p=128), f2sb)

    nc.compile()
    return nc


def _fc0_blk(fc0_w):
    blk = np.zeros((8, 4, 128), np.float32)
    for b in range(BPC):
        blk[2 * b, b, :] = fc0_w[0]
        blk[2 * b + 1, b, :] = fc0_w[1]
    return blk.astype(BF)


def _prep(inputs):
    inp = {k: np.asarray(v) for k, v in inputs.items()}
    F_sb, CH_sb, Ci, grid = _host_consts()
    x = inp["x"].astype(np.float32)  # [32, 8192, 1]
    fc0_w = inp["fc0_w"].astype(np.float32)
    WT = np.stack([inp[f"w{i}_w"].astype(np.float32).T for i in range(4)], 1)
    SW = np.empty((4, 128, MODES, 2, 128), np.float32)
    for i in range(4):
        sw = np.asarray(inp[f"sw{i}"])
        SW[i, :, :, 0, :] = np.ascontiguousarray(sw.real).transpose(0, 2, 1)
        SW[i, :, :, 1, :] = np.ascontiguousarray(sw.imag).transpose(0, 2, 1)
    cg1 = inp["cg1_w"].astype(np.float32).reshape(WIDTH, M_CHEB, L_SEG, 9, 2, 256)
    G2 = cg1.sum(axis=2).reshape(WIDTH, 72, 256)
    lb = np.stack([inp[f"w{i}_b"].astype(np.float32) for i in range(3)], 1)
    common = {
        "fc0w": _fc0_blk(fc0_w),
        "Fb": F_sb, "CHb": CH_sb, "Cinv": Ci,
        "WT": WT.astype(BF),
        "SW": SW.astype(BF),
        "G2": G2.astype(BF),
        "fc1w": inp["fc1_w"].astype(np.float32).astype(BF),
        "fc2w": inp["fc2_w"].astype(np.float32).astype(BF),
        "cg2h": inp["cg2_w"].astype(np.float32).reshape(2, 128, 128)
                .transpose(1, 0, 2).copy().astype(BF),
        "fc0b": inp["fc0_b"].astype(np.float32).reshape(128, 1),
        "lb": lb,
        "w3b": (inp["w3_b"].astype(np.float32)
                + inp["cg2_b"].astype(np.float32)).reshape(128, 1),
        "fc1b": inp["fc1_b"].astype(np.float32).reshape(128, 1),
        "cg1b": np.broadcast_to(inp["cg1_b"].astype(np.float32), (4, 256)).copy(),
    }
    per_core = []
    for c in range(NCORES):
        xg = np.empty((2 * BPC, S), np.float32)
        for b in range(BPC):
            xg[2 * b] = x[c * BPC + b, :, 0]
            xg[2 * b + 1] = grid
        m = dict(common)
        m["xg"] = xg.astype(BF)
        per_core.append(m)
    fc2b = float(inp["fc2_b"].astype(np.float32).reshape(-1)[0])
    return per_core, fc2b


def kernel(**inputs) -> np.ndarray:
    from concourse import bass_utils
    per_core, fc2b = _prep(inputs)
    if "nc" not in _CACHE:
        _CACHE["nc"] = _build()
    nc = _CACHE["nc"]
    res = bass_utils.run_bass_kernel_spmd(nc, per_core, core_ids=list(range(NCORES)))
    out = np.empty((B, S, 1), np.float32)
    for c in range(NCORES):
        out[c * BPC:(c + 1) * BPC, :, 0] = res.results[c]["out"]
    return out + fc2b
